# revision 4
# baseline (speedup 1.0000x reference)
"""Distance-aware comb-pilot interpolator for Trainium2 (8 NeuronCores), v2.

Math per core (batch-sharded 512 rows): out[b, 8k+r, c] =
alpha[r]*H[b,k,c] + gamma[r]*H[b,k+1,c] for k<511; the last 8 subcarriers
use the extrapolated pilot (per-r scalars on H[510], H[511]).

Schedule (measured 67.2us vs the 73.4us baseline; same f32 STT math):
- Coefficients are baked as instruction immediates (program cached per
  decay value): no coef input tensor, no coef DMA, no dependency on it.
- Input loads split across rings: the first 65 pilot columns arrive via
  sync HWDGE (land ~8.3us -> first ACTIVATE at 9.8us vs 11.6 baseline),
  h1 via scalar HWDGE, the rest via gpsimd SWDGE.  First store fires at
  12.6us (baseline: 21.0us).
- All output stores on the sync HWDGE ring (idle engine, FIFO in chunk
  order).  Dual-ring stores measured ~383 GB/s vs ~400+ single-ring.
- Per (chunk, r): tmp = gamma*H[k+1] on ScalarE ACTIVATE, out =
  (H*alpha)+tmp as one DVE scalar_tensor_tensor; the two engines run
  the 8 r-lanes in lockstep (~730ns cadence, both near-saturated).
  Measured dead ends: bf16 compute (the interleaved (k,r,c) output
  pattern drops every DVE op to 1x mode, so bf16 buys nothing) and
  GpSimd tensor_tensor r-lanes (starve SWDGE desc-gen: 97us).
- Last 8 subcarriers per tile: per-(r,c) weight tiles built by gpsimd
  memsets + 3 GpSimd tensor ops, folded into the trailing store chunk.
- Remaining fixed overhead: ~8.7us NEFF preamble (instruction fetch +
  engine bring-up) and ~9us wrapper teardown (mass semaphore clears +
  exit barrier) bracket a ~49us data stream at ~390-420 GB/s.
"""

import sys

import numpy as np

for _p in ("/opt/trn_rl_repo", "/root/.axon_site/_ro/trn_rl_repo"):
    if _p not in sys.path:
        sys.path.append(_p)

import concourse.bass as bass
import concourse.tile as tile
from concourse import bacc, mybir
from concourse.bass_utils import run_bass_kernel_spmd

N_CORES = 8
B, NP, NFFT, SPACING = 4096, 512, 4096, 8
B_LOC = B // N_CORES
NSEG = NP - 1
P = 128
N_BT = B_LOC // P

_PROGRAMS = {}  # decay byte-pattern -> compiled program

C0 = 64  # tile0 leading chunk (pilots 0..C0), loaded via HWDGE
CHUNKS = {
    0: [(0, C0), (C0, 272), (272, NSEG)],
    1: [(0, 272), (272, NSEG)],
    2: [(0, 272), (272, NSEG)],
    3: [(0, 272), (272, 448), (448, NSEG)],
}
# store queue per (tile, chunk_idx): 's' = sync HWDGE, 'g' = gpsimd SWDGE
STORE_Q = {
    (0, 0): "s", (0, 1): "s", (0, 2): "s",
    (1, 0): "s", (1, 1): "s",
    (2, 0): "s", (2, 1): "s",
    (3, 0): "s", (3, 1): "s", (3, 2): "s",
}
# r's computed entirely on GpSimd (broadcast-weight tensor_tensor ops)
# for the non-head chunks -- relieves the ScalarE/DVE r-lockstep cadence.
GPS_RS = ()  # GpSimd TT r-lanes starve SWDGE store desc-gen: measured 97us


def _coefs(decay_param):
    x = np.float32(np.asarray(decay_param).reshape(-1)[0])
    d = np.logaddexp(np.float32(0.0), x, dtype=np.float32)  # softplus
    r = np.arange(SPACING, dtype=np.float32)
    eps = np.float32(1e-12)
    wl = np.exp(-d * r, dtype=np.float32)
    wr = np.exp(-d * (np.float32(SPACING) - r), dtype=np.float32)
    w = wl + wr + eps
    alpha, gamma = wl / w, wr / w
    # last chunk: i = 4088 + r, x0 = 4088, x1 = 4095 (gap 7);
    # y1 = hN = 1.875*H[511] - 0.875*H[510]
    wl2 = np.exp(-d * r, dtype=np.float32)
    wr2 = np.exp(-d * (np.float32(7.0) - r), dtype=np.float32)
    w2 = wl2 + wr2 + eps
    c511 = (wl2 + np.float32(1.875) * wr2) / w2
    c510 = -np.float32(0.875) * wr2 / w2
    return alpha, gamma, c510, c511


def _build_program(alpha, gamma, c510, c511):
    nc = bacc.Bacc("TRN2", target_bir_lowering=False, debug=False,
                   enable_partition_id=False, monotonic_sem_count=0)
    f32 = mybir.dt.float32
    ls = nc.dram_tensor("ls", [B_LOC, NP * 2], f32, kind="ExternalInput").ap()
    out = nc.dram_tensor("out", [B_LOC, NFFT * 2], f32, kind="ExternalOutput").ap()
    mult, add = mybir.AluOpType.mult, mybir.AluOpType.add

    with tile.TileContext(nc) as tc:
        with (
            tc.tile_pool(name="hpool", bufs=4) as hpool,
            tc.tile_pool(name="opool", bufs=3) as opool,
            tc.tile_pool(name="tpool", bufs=8) as tpool,
            tc.tile_pool(name="wpool", bufs=1) as wpool,
            tc.tile_pool(name="gpool", bufs=3) as gpool,
            tc.tile_pool(name="lpool", bufs=2) as lpool,
        ):
            # Dummy first activation: makes bacc place the ACT_TABLE_LOAD
            # here (top of the body) so the ~1.3us table DMA overlaps the
            # input loads instead of gating the first real tmp.
            dm0 = wpool.tile([P, 2], f32, name="dm0")
            nc.vector.memset(dm0[:], 0.0)
            dm1 = wpool.tile([P, 2], f32, name="dm1")
            nc.scalar.mul(dm1[:], dm0[:], 1.0)

            H0A = 2 * C0 + 2  # head columns (pilots 0..C0 inclusive)
            h0a = hpool.tile([P, H0A], f32, name="h0a", tag="h0a")
            ld_hoist = [nc.sync.dma_start(h0a[:], ls[0:P, 0:H0A])]
            h0b = hpool.tile([P, NP * 2 - 2 * C0], f32, name="h0b", tag="h0b")
            ld_hoist.append(nc.gpsimd.dma_start(h0b[:], ls[0:P, 2 * C0 :]))
            hs = [None] + [
                hpool.tile([P, NP * 2], f32, name=f"h{t}", tag="h")
                for t in range(1, N_BT)
            ]
            ld_hoist.append(nc.scalar.dma_start(hs[1][:], ls[P : 2 * P, :]))
            for t in range(2, N_BT):
                ld_hoist.append(
                    nc.gpsimd.dma_start(hs[t][:], ls[t * P : (t + 1) * P, :])
                )

            # last-chunk per-(r,c) weights, built by memsets on gpsimd
            a_last = wpool.tile([P, SPACING, 2], f32)
            c_last = wpool.tile([P, SPACING, 2], f32)
            for r in range(SPACING):
                nc.gpsimd.memset(a_last[:, r, :], float(c510[r]))
                nc.gpsimd.memset(c_last[:, r, :], float(c511[r]))
            # broadcast scalar tiles for the GpSimd r-lanes
            gw = {}
            for r in GPS_RS:
                aw = wpool.tile([P, 2], f32, name=f"aw{r}")
                nc.gpsimd.memset(aw[:], float(alpha[r]))
                cw = wpool.tile([P, 2], f32, name=f"cw{r}")
                nc.gpsimd.memset(cw[:], float(gamma[r]))
                gw[r] = (aw, cw)

            def hseg(t, k0, k1):
                """[P, k1-k0, 2] f32 view of pilots k0..k1 of tile t."""
                if t == 0:
                    if k1 <= C0 + 1:
                        return h0a[:, 2 * k0 : 2 * k1].rearrange(
                            "p (k c) -> p k c", c=2
                        )
                    assert k0 >= C0, (k0, k1)
                    return h0b[:, 2 * (k0 - C0) : 2 * (k1 - C0)].rearrange(
                        "p (k c) -> p k c", c=2
                    )
                return hs[t][:, 2 * k0 : 2 * k1].rearrange("p (k c) -> p k c", c=2)

            for t in range(N_BT):
                o = opool.tile([P, NFFT * 2], f32)
                ov = o[:].rearrange("p (k r c) -> p k r c", r=SPACING, c=2)

                for ci, (k0, k1) in enumerate(CHUNKS[t]):
                    n = k1 - k0
                    last = ci == len(CHUNKS[t]) - 1
                    first0 = t == 0 and ci == 0
                    for r in range(SPACING):
                        if r in GPS_RS and not first0:
                            # whole r-lane on GpSimd: two broadcast-weight
                            # muls + add (TENSOR_TENSOR is Pool-legal)
                            aw, cw = gw[r]
                            awb = aw[:].unsqueeze(1).broadcast_to((P, n, 2))
                            cwb = cw[:].unsqueeze(1).broadcast_to((P, n, 2))
                            t1 = gpool.tile([P, n, 2], f32, name="t1g", tag="t1g")
                            nc.gpsimd.tensor_mul(t1[:], hseg(t, k0, k1), awb)
                            t2 = gpool.tile([P, n, 2], f32, name="t2g", tag="t2g")
                            nc.gpsimd.tensor_mul(
                                t2[:], hseg(t, k0 + 1, k1 + 1), cwb
                            )
                            nc.gpsimd.tensor_add(ov[:, k0:k1, r, :], t1[:], t2[:])
                            continue
                        t2 = tpool.tile([P, n, 2], f32, name="t2", tag="t2")
                        nc.scalar.mul(
                            t2[:], hseg(t, k0 + 1, k1 + 1), float(gamma[r])
                        )
                        nc.vector.scalar_tensor_tensor(
                            ov[:, k0:k1, r, :],
                            hseg(t, k0, k1),
                            float(alpha[r]),
                            t2[:],
                            mult,
                            add,
                        )

                    if last:
                        # subcarriers 4088..4095: per-(r,c) weights on
                        # H[510]/H[511] (GpSimd, off the DVE/ACT path)
                        h510 = hseg(t, NP - 2, NP - 1).broadcast_to((P, SPACING, 2))
                        h511 = hseg(t, NP - 1, NP).broadcast_to((P, SPACING, 2))
                        tl = lpool.tile([P, SPACING, 2], f32)
                        nc.gpsimd.tensor_mul(tl[:], h510, a_last[:])
                        t2l = lpool.tile([P, SPACING, 2], f32)
                        nc.gpsimd.tensor_mul(t2l[:], h511, c_last[:])
                        o_last = o[:, NSEG * 16 : NFFT * 2].rearrange(
                            "p (r c) -> p r c", c=2
                        )
                        nc.gpsimd.tensor_add(o_last, tl[:], t2l[:])

                    lo = k0 * 16
                    hi = NFFT * 2 if last else k1 * 16
                    eng = nc.sync if STORE_Q[(t, ci)] == "s" else nc.gpsimd
                    eng.dma_start(out[t * P : (t + 1) * P, lo:hi], o[:, lo:hi])

    # Hoist the input-load DMA triggers out of the tile-context block into
    # the entry block, ahead of the framework's const-ap memsets and the
    # all-engine barrier.  Each engine's register preamble (which the DMA
    # descriptors need) still precedes them; their completion semaphores
    # are zero at entry (the NEFF wrapper clears the whole sem file before
    # looping).  This starts the input stream ~2.5us earlier.
    main_blk = nc.main_func.blocks[0]
    tc_blk = next(
        b for b in nc.main_func.blocks if b.name.startswith("tile_context")
    )
    moved = []
    for h in ld_hoist:
        i = h.ins
        si = i.sync_info
        if si is not None and getattr(si, "on_wait", None):
            continue  # scheduler attached a wait; not safe to hoist
        tc_blk.instructions.remove(i)
        moved.append(i)
    for pos, i in enumerate(moved, start=1):
        main_blk.instructions.insert(pos, i)

    nc.compile()
    return nc


def kernel(LS_ri, pilot_pos=None, decay_param=None, Nfft=None, **_unused):
    LS_ri = np.ascontiguousarray(np.asarray(LS_ri, dtype=np.float32))
    key = np.float32(np.asarray(decay_param).reshape(-1)[0]).tobytes()
    if key not in _PROGRAMS:
        _PROGRAMS[key] = _build_program(*_coefs(decay_param))
    nc = _PROGRAMS[key]

    in_maps = [
        {"ls": LS_ri[c * B_LOC : (c + 1) * B_LOC].reshape(B_LOC, NP * 2)}
        for c in range(N_CORES)
    ]
    res = run_bass_kernel_spmd(nc, in_maps, list(range(N_CORES))).results
    return np.concatenate(
        [res[c]["out"].reshape(B_LOC, NFFT, 2) for c in range(N_CORES)], axis=0
    )


# revision 6
# speedup vs baseline: 1.0826x; 1.0826x over previous
"""Distance-aware comb-pilot interpolator for Trainium2 (8 NeuronCores), v2.

Math per core (batch-sharded 512 rows): out[b, 8k+r, c] =
alpha[r]*H[b,k,c] + gamma[r]*H[b,k+1,c] for k<511; the last 8 subcarriers
use the extrapolated pilot (per-r scalars on H[510], H[511]).

Schedule (measured 67.2us vs the 73.4us baseline; same f32 STT math):
- Coefficients are baked as instruction immediates (program cached per
  decay value): no coef input tensor, no coef DMA, no dependency on it.
- Input loads split across rings: the first 65 pilot columns arrive via
  sync HWDGE (land ~8.3us -> first ACTIVATE at 9.8us vs 11.6 baseline),
  h1 via scalar HWDGE, the rest via gpsimd SWDGE.  First store fires at
  12.6us (baseline: 21.0us).
- All output stores on the sync HWDGE ring (idle engine, FIFO in chunk
  order).  Dual-ring stores measured ~383 GB/s vs ~400+ single-ring.
- Per (chunk, r): tmp = gamma*H[k+1] on ScalarE ACTIVATE, out =
  (H*alpha)+tmp as one DVE scalar_tensor_tensor; the two engines run
  the 8 r-lanes in lockstep (~730ns cadence, both near-saturated).
  Measured dead ends: bf16 compute (the interleaved (k,r,c) output
  pattern drops every DVE op to 1x mode, so bf16 buys nothing) and
  GpSimd tensor_tensor r-lanes (starve SWDGE desc-gen: 97us).
- Last 8 subcarriers per tile: per-(r,c) weight tiles built by gpsimd
  memsets + 3 GpSimd tensor ops, folded into the trailing store chunk.
- Remaining fixed overhead: ~8.7us NEFF preamble (instruction fetch +
  engine bring-up) and ~9us wrapper teardown (mass semaphore clears +
  exit barrier) bracket a ~49us data stream at ~390-420 GB/s.
"""

import sys

import numpy as np

for _p in ("/opt/trn_rl_repo", "/root/.axon_site/_ro/trn_rl_repo"):
    if _p not in sys.path:
        sys.path.append(_p)

import concourse.bass as bass
import concourse.tile as tile
from concourse import bacc, mybir
from concourse.bass_utils import run_bass_kernel_spmd

N_CORES = 8
B, NP, NFFT, SPACING = 4096, 512, 4096, 8
B_LOC = B // N_CORES
NSEG = NP - 1
P = 128
N_BT = B_LOC // P

_PROGRAMS = {}  # decay byte-pattern -> compiled program

C0 = 64  # tile0 leading chunk (pilots 0..C0), loaded via HWDGE
CHUNKS = {
    0: [(0, C0), (C0, 272), (272, NSEG)],
    1: [(0, 272), (272, NSEG)],
    2: [(0, 272), (272, NSEG)],
    3: [(0, 272), (272, 448), (448, NSEG)],
}
# store queue per (tile, chunk_idx): 's' = sync HWDGE, 'g' = gpsimd SWDGE
STORE_Q = {
    (0, 0): "s", (0, 1): "s", (0, 2): "s",
    (1, 0): "s", (1, 1): "s",
    (2, 0): "s", (2, 1): "s",
    (3, 0): "s", (3, 1): "s", (3, 2): "s",
}
# r's computed entirely on GpSimd (broadcast-weight tensor_tensor ops)
# for the non-head chunks -- relieves the ScalarE/DVE r-lockstep cadence.
GPS_RS = ()  # GpSimd TT r-lanes starve SWDGE store desc-gen: measured 97us


def _coefs(decay_param):
    x = np.float32(np.asarray(decay_param).reshape(-1)[0])
    d = np.logaddexp(np.float32(0.0), x, dtype=np.float32)  # softplus
    r = np.arange(SPACING, dtype=np.float32)
    eps = np.float32(1e-12)
    wl = np.exp(-d * r, dtype=np.float32)
    wr = np.exp(-d * (np.float32(SPACING) - r), dtype=np.float32)
    w = wl + wr + eps
    alpha, gamma = wl / w, wr / w
    # last chunk: i = 4088 + r, x0 = 4088, x1 = 4095 (gap 7);
    # y1 = hN = 1.875*H[511] - 0.875*H[510]
    wl2 = np.exp(-d * r, dtype=np.float32)
    wr2 = np.exp(-d * (np.float32(7.0) - r), dtype=np.float32)
    w2 = wl2 + wr2 + eps
    c511 = (wl2 + np.float32(1.875) * wr2) / w2
    c510 = -np.float32(0.875) * wr2 / w2
    return alpha, gamma, c510, c511


def _build_program(alpha, gamma, c510, c511):
    nc = bacc.Bacc("TRN2", target_bir_lowering=False, debug=False,
                   enable_partition_id=False, monotonic_sem_count=0)
    f32 = mybir.dt.float32
    ls = nc.dram_tensor("ls", [B_LOC, NP * 2], f32, kind="ExternalInput").ap()
    out = nc.dram_tensor("out", [B_LOC, NFFT * 2], f32, kind="ExternalOutput").ap()
    mult, add = mybir.AluOpType.mult, mybir.AluOpType.add

    with tile.TileContext(nc) as tc:
        with (
            tc.tile_pool(name="hpool", bufs=4) as hpool,
            tc.tile_pool(name="opool", bufs=3) as opool,
            tc.tile_pool(name="tpool", bufs=8) as tpool,
            tc.tile_pool(name="wpool", bufs=1) as wpool,
            tc.tile_pool(name="gpool", bufs=3) as gpool,
            tc.tile_pool(name="lpool", bufs=2) as lpool,
        ):
            H0A = 2 * C0 + 2  # head columns (pilots 0..C0 inclusive)
            h0a = hpool.tile([P, H0A], f32, name="h0a", tag="h0a")
            nc.sync.dma_start(h0a[:], ls[0:P, 0:H0A])
            h0b = hpool.tile([P, NP * 2 - 2 * C0], f32, name="h0b", tag="h0b")
            nc.gpsimd.dma_start(h0b[:], ls[0:P, 2 * C0 :])
            hs = [None] + [
                hpool.tile([P, NP * 2], f32, name=f"h{t}", tag="h")
                for t in range(1, N_BT)
            ]
            nc.scalar.dma_start(hs[1][:], ls[P : 2 * P, :])
            for t in range(2, N_BT):
                nc.gpsimd.dma_start(hs[t][:], ls[t * P : (t + 1) * P, :])

            # last-chunk per-(r,c) weights, built by memsets on gpsimd
            a_last = wpool.tile([P, SPACING, 2], f32)
            c_last = wpool.tile([P, SPACING, 2], f32)
            for r in range(SPACING):
                nc.gpsimd.memset(a_last[:, r, :], float(c510[r]))
                nc.gpsimd.memset(c_last[:, r, :], float(c511[r]))
            # broadcast scalar tiles for the GpSimd r-lanes
            gw = {}
            for r in GPS_RS:
                aw = wpool.tile([P, 2], f32, name=f"aw{r}")
                nc.gpsimd.memset(aw[:], float(alpha[r]))
                cw = wpool.tile([P, 2], f32, name=f"cw{r}")
                nc.gpsimd.memset(cw[:], float(gamma[r]))
                gw[r] = (aw, cw)

            def hseg(t, k0, k1):
                """[P, k1-k0, 2] f32 view of pilots k0..k1 of tile t."""
                if t == 0:
                    if k1 <= C0 + 1:
                        return h0a[:, 2 * k0 : 2 * k1].rearrange(
                            "p (k c) -> p k c", c=2
                        )
                    assert k0 >= C0, (k0, k1)
                    return h0b[:, 2 * (k0 - C0) : 2 * (k1 - C0)].rearrange(
                        "p (k c) -> p k c", c=2
                    )
                return hs[t][:, 2 * k0 : 2 * k1].rearrange("p (k c) -> p k c", c=2)

            for t in range(N_BT):
                o = opool.tile([P, NFFT * 2], f32)
                ov = o[:].rearrange("p (k r c) -> p k r c", r=SPACING, c=2)

                for ci, (k0, k1) in enumerate(CHUNKS[t]):
                    n = k1 - k0
                    last = ci == len(CHUNKS[t]) - 1
                    first0 = t == 0 and ci == 0
                    for r in range(SPACING):
                        if r in GPS_RS and not first0:
                            # whole r-lane on GpSimd: two broadcast-weight
                            # muls + add (TENSOR_TENSOR is Pool-legal)
                            aw, cw = gw[r]
                            awb = aw[:].unsqueeze(1).broadcast_to((P, n, 2))
                            cwb = cw[:].unsqueeze(1).broadcast_to((P, n, 2))
                            t1 = gpool.tile([P, n, 2], f32, name="t1g", tag="t1g")
                            nc.gpsimd.tensor_mul(t1[:], hseg(t, k0, k1), awb)
                            t2 = gpool.tile([P, n, 2], f32, name="t2g", tag="t2g")
                            nc.gpsimd.tensor_mul(
                                t2[:], hseg(t, k0 + 1, k1 + 1), cwb
                            )
                            nc.gpsimd.tensor_add(ov[:, k0:k1, r, :], t1[:], t2[:])
                            continue
                        t2 = tpool.tile([P, n, 2], f32, name="t2", tag="t2")
                        nc.scalar.mul(
                            t2[:], hseg(t, k0 + 1, k1 + 1), float(gamma[r])
                        )
                        nc.vector.scalar_tensor_tensor(
                            ov[:, k0:k1, r, :],
                            hseg(t, k0, k1),
                            float(alpha[r]),
                            t2[:],
                            mult,
                            add,
                        )

                    if last:
                        # subcarriers 4088..4095: per-(r,c) weights on
                        # H[510]/H[511] (GpSimd, off the DVE/ACT path)
                        h510 = hseg(t, NP - 2, NP - 1).broadcast_to((P, SPACING, 2))
                        h511 = hseg(t, NP - 1, NP).broadcast_to((P, SPACING, 2))
                        tl = lpool.tile([P, SPACING, 2], f32)
                        nc.gpsimd.tensor_mul(tl[:], h510, a_last[:])
                        t2l = lpool.tile([P, SPACING, 2], f32)
                        nc.gpsimd.tensor_mul(t2l[:], h511, c_last[:])
                        o_last = o[:, NSEG * 16 : NFFT * 2].rearrange(
                            "p (r c) -> p r c", c=2
                        )
                        nc.gpsimd.tensor_add(o_last, tl[:], t2l[:])

                    lo = k0 * 16
                    hi = NFFT * 2 if last else k1 * 16
                    eng = nc.sync if STORE_Q[(t, ci)] == "s" else nc.gpsimd
                    eng.dma_start(out[t * P : (t + 1) * P, lo:hi], o[:, lo:hi])
    # (measured dead end: hoisting the load DMA triggers into the entry
    # block ahead of the init barrier made every run ~6us SLOWER -- the
    # early data DMAs contend with the NEFF instruction-stream fetch and
    # delay engine bring-up.)
    nc.compile()
    return nc


def kernel(LS_ri, pilot_pos=None, decay_param=None, Nfft=None, **_unused):
    LS_ri = np.ascontiguousarray(np.asarray(LS_ri, dtype=np.float32))
    key = np.float32(np.asarray(decay_param).reshape(-1)[0]).tobytes()
    if key not in _PROGRAMS:
        _PROGRAMS[key] = _build_program(*_coefs(decay_param))
    nc = _PROGRAMS[key]

    in_maps = [
        {"ls": LS_ri[c * B_LOC : (c + 1) * B_LOC].reshape(B_LOC, NP * 2)}
        for c in range(N_CORES)
    ]
    res = run_bass_kernel_spmd(nc, in_maps, list(range(N_CORES))).results
    return np.concatenate(
        [res[c]["out"].reshape(B_LOC, NFFT, 2) for c in range(N_CORES)], axis=0
    )


# revision 7
# speedup vs baseline: 1.0882x; 1.0052x over previous
"""Distance-aware comb-pilot interpolator for Trainium2 (8 NeuronCores), v2.

Math per core (batch-sharded 512 rows): out[b, 8k+r, c] =
alpha[r]*H[b,k,c] + gamma[r]*H[b,k+1,c] for k<511; the last 8 subcarriers
use the extrapolated pilot (per-r scalars on H[510], H[511]).

Schedule (measured 67.2us vs the 73.4us baseline; same f32 STT math):
- Coefficients are baked as instruction immediates (program cached per
  decay value): no coef input tensor, no coef DMA, no dependency on it.
- Input loads split across rings: the first 65 pilot columns arrive via
  sync HWDGE (land ~8.3us -> first ACTIVATE at 9.8us vs 11.6 baseline),
  h1 via scalar HWDGE, the rest via gpsimd SWDGE.  First store fires at
  12.6us (baseline: 21.0us).
- All output stores on the sync HWDGE ring (idle engine, FIFO in chunk
  order).  Dual-ring stores measured ~383 GB/s vs ~400+ single-ring.
- Per (chunk, r): tmp = gamma*H[k+1] on ScalarE ACTIVATE, out =
  (H*alpha)+tmp as one DVE scalar_tensor_tensor; the two engines run
  the 8 r-lanes in lockstep (~730ns cadence, both near-saturated).
  Measured dead ends: bf16 compute (the interleaved (k,r,c) output
  pattern drops every DVE op to 1x mode, so bf16 buys nothing) and
  GpSimd tensor_tensor r-lanes (starve SWDGE desc-gen: 97us).
- Last 8 subcarriers per tile: per-(r,c) weight tiles built by gpsimd
  memsets + 3 GpSimd tensor ops, folded into the trailing store chunk.
- Remaining fixed overhead: ~8.7us NEFF preamble (instruction fetch +
  engine bring-up) and ~9us wrapper teardown (mass semaphore clears +
  exit barrier) bracket a ~49us data stream at ~390-420 GB/s.
"""

import sys

import numpy as np

for _p in ("/opt/trn_rl_repo", "/root/.axon_site/_ro/trn_rl_repo"):
    if _p not in sys.path:
        sys.path.append(_p)

import concourse.bass as bass
import concourse.tile as tile
from concourse import bacc, mybir
from concourse.bass_utils import run_bass_kernel_spmd

N_CORES = 8
B, NP, NFFT, SPACING = 4096, 512, 4096, 8
B_LOC = B // N_CORES
NSEG = NP - 1
P = 128
N_BT = B_LOC // P

_PROGRAMS = {}  # decay byte-pattern -> compiled program

C0 = 64  # tile0 leading chunk (pilots 0..C0), loaded via HWDGE
CHUNKS = {
    0: [(0, C0), (C0, 320), (320, NSEG)],
    1: [(0, 272), (272, NSEG)],
    2: [(0, 272), (272, NSEG)],
    3: [(0, 272), (272, 448), (448, NSEG)],
}
# store queue per (tile, chunk_idx): 's' = sync HWDGE, 'g' = gpsimd SWDGE
STORE_Q = {
    (0, 0): "s", (0, 1): "s", (0, 2): "s",
    (1, 0): "s", (1, 1): "s",
    (2, 0): "s", (2, 1): "s",
    (3, 0): "s", (3, 1): "s", (3, 2): "s",
}
# r's computed entirely on GpSimd (broadcast-weight tensor_tensor ops)
# for the non-head chunks -- relieves the ScalarE/DVE r-lockstep cadence.
GPS_RS = ()  # GpSimd TT r-lanes starve SWDGE store desc-gen: measured 97us


def _coefs(decay_param):
    x = np.float32(np.asarray(decay_param).reshape(-1)[0])
    d = np.logaddexp(np.float32(0.0), x, dtype=np.float32)  # softplus
    r = np.arange(SPACING, dtype=np.float32)
    eps = np.float32(1e-12)
    wl = np.exp(-d * r, dtype=np.float32)
    wr = np.exp(-d * (np.float32(SPACING) - r), dtype=np.float32)
    w = wl + wr + eps
    alpha, gamma = wl / w, wr / w
    # last chunk: i = 4088 + r, x0 = 4088, x1 = 4095 (gap 7);
    # y1 = hN = 1.875*H[511] - 0.875*H[510]
    wl2 = np.exp(-d * r, dtype=np.float32)
    wr2 = np.exp(-d * (np.float32(7.0) - r), dtype=np.float32)
    w2 = wl2 + wr2 + eps
    c511 = (wl2 + np.float32(1.875) * wr2) / w2
    c510 = -np.float32(0.875) * wr2 / w2
    return alpha, gamma, c510, c511


def _build_program(alpha, gamma, c510, c511):
    nc = bacc.Bacc("TRN2", target_bir_lowering=False, debug=False,
                   enable_partition_id=False, monotonic_sem_count=0)
    f32 = mybir.dt.float32
    ls = nc.dram_tensor("ls", [B_LOC, NP * 2], f32, kind="ExternalInput").ap()
    out = nc.dram_tensor("out", [B_LOC, NFFT * 2], f32, kind="ExternalOutput").ap()
    mult, add = mybir.AluOpType.mult, mybir.AluOpType.add

    with tile.TileContext(nc) as tc:
        with (
            tc.tile_pool(name="hpool", bufs=4) as hpool,
            tc.tile_pool(name="opool", bufs=3) as opool,
            tc.tile_pool(name="tpool", bufs=8) as tpool,
            tc.tile_pool(name="wpool", bufs=1) as wpool,
            tc.tile_pool(name="gpool", bufs=3) as gpool,
            tc.tile_pool(name="lpool", bufs=2) as lpool,
        ):
            H0A = 2 * C0 + 2  # head columns (pilots 0..C0 inclusive)
            h0a = hpool.tile([P, H0A], f32, name="h0a", tag="h0a")
            nc.sync.dma_start(h0a[:], ls[0:P, 0:H0A])
            h0b = hpool.tile([P, NP * 2 - 2 * C0], f32, name="h0b", tag="h0b")
            nc.gpsimd.dma_start(h0b[:], ls[0:P, 2 * C0 :])
            hs = [None] + [
                hpool.tile([P, NP * 2], f32, name=f"h{t}", tag="h")
                for t in range(1, N_BT)
            ]
            nc.scalar.dma_start(hs[1][:], ls[P : 2 * P, :])
            for t in range(2, N_BT):
                nc.gpsimd.dma_start(hs[t][:], ls[t * P : (t + 1) * P, :])

            # last-chunk per-(r,c) weights, built by memsets on gpsimd
            a_last = wpool.tile([P, SPACING, 2], f32)
            c_last = wpool.tile([P, SPACING, 2], f32)
            for r in range(SPACING):
                nc.gpsimd.memset(a_last[:, r, :], float(c510[r]))
                nc.gpsimd.memset(c_last[:, r, :], float(c511[r]))
            # broadcast scalar tiles for the GpSimd r-lanes
            gw = {}
            for r in GPS_RS:
                aw = wpool.tile([P, 2], f32, name=f"aw{r}")
                nc.gpsimd.memset(aw[:], float(alpha[r]))
                cw = wpool.tile([P, 2], f32, name=f"cw{r}")
                nc.gpsimd.memset(cw[:], float(gamma[r]))
                gw[r] = (aw, cw)

            def hseg(t, k0, k1):
                """[P, k1-k0, 2] f32 view of pilots k0..k1 of tile t."""
                if t == 0:
                    if k1 <= C0 + 1:
                        return h0a[:, 2 * k0 : 2 * k1].rearrange(
                            "p (k c) -> p k c", c=2
                        )
                    assert k0 >= C0, (k0, k1)
                    return h0b[:, 2 * (k0 - C0) : 2 * (k1 - C0)].rearrange(
                        "p (k c) -> p k c", c=2
                    )
                return hs[t][:, 2 * k0 : 2 * k1].rearrange("p (k c) -> p k c", c=2)

            for t in range(N_BT):
                o = opool.tile([P, NFFT * 2], f32)
                ov = o[:].rearrange("p (k r c) -> p k r c", r=SPACING, c=2)

                for ci, (k0, k1) in enumerate(CHUNKS[t]):
                    n = k1 - k0
                    last = ci == len(CHUNKS[t]) - 1
                    first0 = t == 0 and ci == 0
                    for r in range(SPACING):
                        if r in GPS_RS and not first0:
                            # whole r-lane on GpSimd: two broadcast-weight
                            # muls + add (TENSOR_TENSOR is Pool-legal)
                            aw, cw = gw[r]
                            awb = aw[:].unsqueeze(1).broadcast_to((P, n, 2))
                            cwb = cw[:].unsqueeze(1).broadcast_to((P, n, 2))
                            t1 = gpool.tile([P, n, 2], f32, name="t1g", tag="t1g")
                            nc.gpsimd.tensor_mul(t1[:], hseg(t, k0, k1), awb)
                            t2 = gpool.tile([P, n, 2], f32, name="t2g", tag="t2g")
                            nc.gpsimd.tensor_mul(
                                t2[:], hseg(t, k0 + 1, k1 + 1), cwb
                            )
                            nc.gpsimd.tensor_add(ov[:, k0:k1, r, :], t1[:], t2[:])
                            continue
                        t2 = tpool.tile([P, n, 2], f32, name="t2", tag="t2")
                        nc.scalar.mul(
                            t2[:], hseg(t, k0 + 1, k1 + 1), float(gamma[r])
                        )
                        nc.vector.scalar_tensor_tensor(
                            ov[:, k0:k1, r, :],
                            hseg(t, k0, k1),
                            float(alpha[r]),
                            t2[:],
                            mult,
                            add,
                        )

                    if last:
                        # subcarriers 4088..4095: per-(r,c) weights on
                        # H[510]/H[511] (GpSimd, off the DVE/ACT path)
                        h510 = hseg(t, NP - 2, NP - 1).broadcast_to((P, SPACING, 2))
                        h511 = hseg(t, NP - 1, NP).broadcast_to((P, SPACING, 2))
                        tl = lpool.tile([P, SPACING, 2], f32)
                        nc.gpsimd.tensor_mul(tl[:], h510, a_last[:])
                        t2l = lpool.tile([P, SPACING, 2], f32)
                        nc.gpsimd.tensor_mul(t2l[:], h511, c_last[:])
                        o_last = o[:, NSEG * 16 : NFFT * 2].rearrange(
                            "p (r c) -> p r c", c=2
                        )
                        nc.gpsimd.tensor_add(o_last, tl[:], t2l[:])

                    lo = k0 * 16
                    hi = NFFT * 2 if last else k1 * 16
                    eng = nc.sync if STORE_Q[(t, ci)] == "s" else nc.gpsimd
                    eng.dma_start(out[t * P : (t + 1) * P, lo:hi], o[:, lo:hi])
    # (measured dead end: hoisting the load DMA triggers into the entry
    # block ahead of the init barrier made every run ~6us SLOWER -- the
    # early data DMAs contend with the NEFF instruction-stream fetch and
    # delay engine bring-up.)
    nc.compile()
    return nc


def kernel(LS_ri, pilot_pos=None, decay_param=None, Nfft=None, **_unused):
    LS_ri = np.ascontiguousarray(np.asarray(LS_ri, dtype=np.float32))
    key = np.float32(np.asarray(decay_param).reshape(-1)[0]).tobytes()
    if key not in _PROGRAMS:
        _PROGRAMS[key] = _build_program(*_coefs(decay_param))
    nc = _PROGRAMS[key]

    in_maps = [
        {"ls": LS_ri[c * B_LOC : (c + 1) * B_LOC].reshape(B_LOC, NP * 2)}
        for c in range(N_CORES)
    ]
    res = run_bass_kernel_spmd(nc, in_maps, list(range(N_CORES))).results
    return np.concatenate(
        [res[c]["out"].reshape(B_LOC, NFFT, 2) for c in range(N_CORES)], axis=0
    )


# revision 8
# speedup vs baseline: 1.0922x; 1.0036x over previous
"""Distance-aware comb-pilot interpolator for Trainium2 (8 NeuronCores), v2.

Math per core (batch-sharded 512 rows): out[b, 8k+r, c] =
alpha[r]*H[b,k,c] + gamma[r]*H[b,k+1,c] for k<511; the last 8 subcarriers
use the extrapolated pilot (per-r scalars on H[510], H[511]).

Schedule (measured 67.2us vs the 73.4us baseline; same f32 STT math):
- Coefficients are baked as instruction immediates (program cached per
  decay value): no coef input tensor, no coef DMA, no dependency on it.
- Input loads split across rings: the first 65 pilot columns arrive via
  sync HWDGE (land ~8.3us -> first ACTIVATE at 9.8us vs 11.6 baseline),
  h1 via scalar HWDGE, the rest via gpsimd SWDGE.  First store fires at
  12.6us (baseline: 21.0us).
- All output stores on the sync HWDGE ring (idle engine, FIFO in chunk
  order).  Dual-ring stores measured ~383 GB/s vs ~400+ single-ring.
- Per (chunk, r): tmp = gamma*H[k+1] on ScalarE ACTIVATE, out =
  (H*alpha)+tmp as one DVE scalar_tensor_tensor; the two engines run
  the 8 r-lanes in lockstep (~730ns cadence, both near-saturated).
  Measured dead ends: bf16 compute (the interleaved (k,r,c) output
  pattern drops every DVE op to 1x mode, so bf16 buys nothing) and
  GpSimd tensor_tensor r-lanes (starve SWDGE desc-gen: 97us).
- Last 8 subcarriers per tile: per-(r,c) weight tiles built by gpsimd
  memsets + 3 GpSimd tensor ops, folded into the trailing store chunk.
- Remaining fixed overhead: ~8.7us NEFF preamble (instruction fetch +
  engine bring-up) and ~9us wrapper teardown (mass semaphore clears +
  exit barrier) bracket a ~49us data stream at ~390-420 GB/s.
"""

import sys

import numpy as np

for _p in ("/opt/trn_rl_repo", "/root/.axon_site/_ro/trn_rl_repo"):
    if _p not in sys.path:
        sys.path.append(_p)

import concourse.bass as bass
import concourse.tile as tile
from concourse import bacc, mybir
from concourse.bass_utils import run_bass_kernel_spmd

N_CORES = 8
B, NP, NFFT, SPACING = 4096, 512, 4096, 8
B_LOC = B // N_CORES
NSEG = NP - 1
P = 128
N_BT = B_LOC // P

_PROGRAMS = {}  # decay byte-pattern -> compiled program

C0 = 64  # tile0 leading chunk (pilots 0..C0), loaded via HWDGE
CHUNKS = {
    0: [(0, C0), (C0, 320), (320, NSEG)],
    1: [(0, 272), (272, NSEG)],
    2: [(0, 272), (272, NSEG)],
    3: [(0, 272), (272, 448), (448, NSEG)],
}
# store queue per (tile, chunk_idx): 's' = sync HWDGE, 'g' = gpsimd SWDGE
STORE_Q = {
    (0, 0): "s", (0, 1): "s", (0, 2): "s",
    (1, 0): "s", (1, 1): "s",
    (2, 0): "s", (2, 1): "s",
    (3, 0): "s", (3, 1): "s", (3, 2): "s",
}
# r's computed entirely on GpSimd (broadcast-weight tensor_tensor ops)
# for the non-head chunks -- relieves the ScalarE/DVE r-lockstep cadence.
GPS_RS = ()  # GpSimd TT r-lanes starve SWDGE store desc-gen: measured 97us


def _coefs(decay_param):
    x = np.float32(np.asarray(decay_param).reshape(-1)[0])
    d = np.logaddexp(np.float32(0.0), x, dtype=np.float32)  # softplus
    r = np.arange(SPACING, dtype=np.float32)
    eps = np.float32(1e-12)
    wl = np.exp(-d * r, dtype=np.float32)
    wr = np.exp(-d * (np.float32(SPACING) - r), dtype=np.float32)
    w = wl + wr + eps
    alpha, gamma = wl / w, wr / w
    # last chunk: i = 4088 + r, x0 = 4088, x1 = 4095 (gap 7);
    # y1 = hN = 1.875*H[511] - 0.875*H[510]
    wl2 = np.exp(-d * r, dtype=np.float32)
    wr2 = np.exp(-d * (np.float32(7.0) - r), dtype=np.float32)
    w2 = wl2 + wr2 + eps
    c511 = (wl2 + np.float32(1.875) * wr2) / w2
    c510 = -np.float32(0.875) * wr2 / w2
    return alpha, gamma, c510, c511


def _build_program(alpha, gamma, c510, c511):
    nc = bacc.Bacc("TRN2", target_bir_lowering=False, debug=False,
                   enable_partition_id=False, monotonic_sem_count=0)
    f32 = mybir.dt.float32
    ls = nc.dram_tensor("ls", [B_LOC, NP * 2], f32, kind="ExternalInput").ap()
    out = nc.dram_tensor("out", [B_LOC, NFFT * 2], f32, kind="ExternalOutput").ap()
    mult, add = mybir.AluOpType.mult, mybir.AluOpType.add

    with tile.TileContext(nc) as tc:
        with (
            tc.tile_pool(name="hpool", bufs=4) as hpool,
            tc.tile_pool(name="opool", bufs=3) as opool,
            tc.tile_pool(name="tpool", bufs=8) as tpool,
            tc.tile_pool(name="wpool", bufs=1) as wpool,
            tc.tile_pool(name="gpool", bufs=3) as gpool,
            tc.tile_pool(name="lpool", bufs=2) as lpool,
        ):
            H0A = 2 * C0 + 2  # head columns (pilots 0..C0 inclusive)
            h0a = hpool.tile([P, H0A], f32, name="h0a", tag="h0a")
            nc.sync.dma_start(h0a[:], ls[0:P, 0:H0A])
            h0b = hpool.tile([P, NP * 2 - 2 * C0], f32, name="h0b", tag="h0b")
            nc.gpsimd.dma_start(h0b[:], ls[0:P, 2 * C0 :])
            hs = [None] + [
                hpool.tile([P, NP * 2], f32, name=f"h{t}", tag="h")
                for t in range(1, N_BT)
            ]
            # h1..h3 all on SWDGE behind h0b: an early h1 on the scalar
            # HWDGE ring competes with h0a for SDMA bandwidth and delays
            # the first ACTIVATE by ~1us; h1 is not needed until ~17us.
            for t in range(1, N_BT):
                nc.gpsimd.dma_start(hs[t][:], ls[t * P : (t + 1) * P, :])

            # last-chunk per-(r,c) weights, built by memsets on gpsimd
            a_last = wpool.tile([P, SPACING, 2], f32)
            c_last = wpool.tile([P, SPACING, 2], f32)
            for r in range(SPACING):
                nc.gpsimd.memset(a_last[:, r, :], float(c510[r]))
                nc.gpsimd.memset(c_last[:, r, :], float(c511[r]))
            # broadcast scalar tiles for the GpSimd r-lanes
            gw = {}
            for r in GPS_RS:
                aw = wpool.tile([P, 2], f32, name=f"aw{r}")
                nc.gpsimd.memset(aw[:], float(alpha[r]))
                cw = wpool.tile([P, 2], f32, name=f"cw{r}")
                nc.gpsimd.memset(cw[:], float(gamma[r]))
                gw[r] = (aw, cw)

            def hseg(t, k0, k1):
                """[P, k1-k0, 2] f32 view of pilots k0..k1 of tile t."""
                if t == 0:
                    if k1 <= C0 + 1:
                        return h0a[:, 2 * k0 : 2 * k1].rearrange(
                            "p (k c) -> p k c", c=2
                        )
                    assert k0 >= C0, (k0, k1)
                    return h0b[:, 2 * (k0 - C0) : 2 * (k1 - C0)].rearrange(
                        "p (k c) -> p k c", c=2
                    )
                return hs[t][:, 2 * k0 : 2 * k1].rearrange("p (k c) -> p k c", c=2)

            for t in range(N_BT):
                o = opool.tile([P, NFFT * 2], f32)
                ov = o[:].rearrange("p (k r c) -> p k r c", r=SPACING, c=2)

                for ci, (k0, k1) in enumerate(CHUNKS[t]):
                    n = k1 - k0
                    last = ci == len(CHUNKS[t]) - 1
                    first0 = t == 0 and ci == 0
                    for r in range(SPACING):
                        if r in GPS_RS and not first0:
                            # whole r-lane on GpSimd: two broadcast-weight
                            # muls + add (TENSOR_TENSOR is Pool-legal)
                            aw, cw = gw[r]
                            awb = aw[:].unsqueeze(1).broadcast_to((P, n, 2))
                            cwb = cw[:].unsqueeze(1).broadcast_to((P, n, 2))
                            t1 = gpool.tile([P, n, 2], f32, name="t1g", tag="t1g")
                            nc.gpsimd.tensor_mul(t1[:], hseg(t, k0, k1), awb)
                            t2 = gpool.tile([P, n, 2], f32, name="t2g", tag="t2g")
                            nc.gpsimd.tensor_mul(
                                t2[:], hseg(t, k0 + 1, k1 + 1), cwb
                            )
                            nc.gpsimd.tensor_add(ov[:, k0:k1, r, :], t1[:], t2[:])
                            continue
                        t2 = tpool.tile([P, n, 2], f32, name="t2", tag="t2")
                        nc.scalar.mul(
                            t2[:], hseg(t, k0 + 1, k1 + 1), float(gamma[r])
                        )
                        nc.vector.scalar_tensor_tensor(
                            ov[:, k0:k1, r, :],
                            hseg(t, k0, k1),
                            float(alpha[r]),
                            t2[:],
                            mult,
                            add,
                        )

                    if last:
                        # subcarriers 4088..4095: per-(r,c) weights on
                        # H[510]/H[511] (GpSimd, off the DVE/ACT path)
                        h510 = hseg(t, NP - 2, NP - 1).broadcast_to((P, SPACING, 2))
                        h511 = hseg(t, NP - 1, NP).broadcast_to((P, SPACING, 2))
                        tl = lpool.tile([P, SPACING, 2], f32)
                        nc.gpsimd.tensor_mul(tl[:], h510, a_last[:])
                        t2l = lpool.tile([P, SPACING, 2], f32)
                        nc.gpsimd.tensor_mul(t2l[:], h511, c_last[:])
                        o_last = o[:, NSEG * 16 : NFFT * 2].rearrange(
                            "p (r c) -> p r c", c=2
                        )
                        nc.gpsimd.tensor_add(o_last, tl[:], t2l[:])

                    lo = k0 * 16
                    hi = NFFT * 2 if last else k1 * 16
                    eng = nc.sync if STORE_Q[(t, ci)] == "s" else nc.gpsimd
                    eng.dma_start(out[t * P : (t + 1) * P, lo:hi], o[:, lo:hi])
    # (measured dead end: hoisting the load DMA triggers into the entry
    # block ahead of the init barrier made every run ~6us SLOWER -- the
    # early data DMAs contend with the NEFF instruction-stream fetch and
    # delay engine bring-up.)
    nc.compile()
    return nc


def kernel(LS_ri, pilot_pos=None, decay_param=None, Nfft=None, **_unused):
    LS_ri = np.ascontiguousarray(np.asarray(LS_ri, dtype=np.float32))
    key = np.float32(np.asarray(decay_param).reshape(-1)[0]).tobytes()
    if key not in _PROGRAMS:
        _PROGRAMS[key] = _build_program(*_coefs(decay_param))
    nc = _PROGRAMS[key]

    in_maps = [
        {"ls": LS_ri[c * B_LOC : (c + 1) * B_LOC].reshape(B_LOC, NP * 2)}
        for c in range(N_CORES)
    ]
    res = run_bass_kernel_spmd(nc, in_maps, list(range(N_CORES))).results
    return np.concatenate(
        [res[c]["out"].reshape(B_LOC, NFFT, 2) for c in range(N_CORES)], axis=0
    )
